# revision 30
# baseline (speedup 1.0000x reference)
"""Multi-head self-attention (RoPE) Trainium2 Bass kernel.

Shards batch (B=8) across 8 NeuronCores, one batch element per core.

Design notes (HW-A/B-validated):
- Head-PAIRED scores: q/k for heads (2p, 2p+1) live in the two 64-row
  halves of one qkTp block; the two K=64 score matmuls target disjoint
  PE row groups (tile_position (0,0)/(64,0)) and run concurrently —
  measured ~2x scores throughput vs serial K=128 (22us vs 56us/rep for
  the scores-only phases).
- ScalarE is the critical engine (exp is ACT-only at 1x rate,
  ~(N+352)/1.2GHz): exp runs as 64x [128,1024] ACTs off double-buffered
  2-bank score tiles so PE is never gated on a full-group exp.  PSUM
  drains stay on ScalarE — moving them to VectorE measured WORSE
  (RoPE/softmax latency chains live on DVE).
- All PSUM pools are <=4 banks (2-bank tiles, bufs=2) so adjacent
  phases coexist in PSUM; full-8-bank pools cost ~27us/rep in
  phase-boundary stalls.
- RoPE is pipelined per m-block: ACT drain -> SBUF->SBUF DMA swizzle
  -> DVE mul/mul/add, overlapping the next block's matmuls.
- Softmax denominators come free as ones-columns in the augmented V
  (output-partition waste only); one [128,L] reciprocal serves a whole
  head pair.
"""
import os
import sys

# The kernel needs the 8 axon-tunneled NeuronCores visible to jax; a
# JAX_PLATFORMS=cpu pin (used by some harnesses for the reference) would
# hide them. Clear it before jax initializes through the concourse imports.
os.environ.pop("JAX_PLATFORMS", None)

sys.path.insert(0, "/opt/trn_rl_repo")

_REPS = int(os.environ.get("KREPS", "1"))
_DRAINS = os.environ.get("KDRAINS", "act")  # act|dve: engine for PSUM drains
_PH = int(os.environ.get("KPHASES", "4"))  # 1=QKV 2=+scores 3=+AV 4=full
_NOEXP = int(os.environ.get("KNOEXP", "0"))  # 1: skip exp ACTs (cost attribution only)
_PAIRPOS = int(os.environ.get("KPAIRPOS", "1"))  # 0: both score MMs on row group 0 (timing control)

import numpy as np
from contextlib import ExitStack

import concourse.bass as bass
import concourse.tile as tile
from concourse import bacc, mybir

f32 = mybir.dt.float32
f16 = mybir.dt.float16
AF = mybir.ActivationFunctionType
ALU = mybir.AluOpType

B, L, DIM = 8, 1024, 512
NH, HD = 8, 64
SCALE = HD ** -0.5
NCORES = 8


def _build_nc():
    nc = bacc.Bacc("TRN2", target_bir_lowering=False, debug=False, enable_asserts=False)

    xT = nc.dram_tensor("xT", (DIM, L), f16, kind="ExternalInput")
    wq = nc.dram_tensor("wq", (DIM, 2 * DIM), f16, kind="ExternalInput")  # Q|K cols
    wv = nc.dram_tensor("wv", (DIM, DIM), f16, kind="ExternalInput")      # V cols
    wp = nc.dram_tensor("wp", (DIM, DIM), f16, kind="ExternalInput")
    cosT = nc.dram_tensor("cosT", (128, 8 * L), f16, kind="ExternalInput")
    sinT = nc.dram_tensor("sinT", (128, 8 * L), f16, kind="ExternalInput")
    bias = nc.dram_tensor("bias", (128, 8), f32, kind="ExternalInput")
    y = nc.dram_tensor("y", (L, DIM), f32, kind="ExternalOutput")

    with ExitStack() as ctx:
        tc = ctx.enter_context(tile.TileContext(nc))
        cst = ctx.enter_context(tc.tile_pool(name="cst", bufs=1))
        sc = ctx.enter_context(tc.tile_pool(name="sc", bufs=2))
        pTp = ctx.enter_context(tc.tile_pool(name="pTp", bufs=8))
        ysb = ctx.enter_context(tc.tile_pool(name="ysb", bufs=1))

        # ---- load inputs (once) ----
        xT_all = cst.tile([128, 4 * L], f16, name="t", tag="xTall")
        wq_all = cst.tile([128, 4 * 2 * DIM], f16, name="t", tag="wqall")
        wv_all = cst.tile([128, 4 * DIM], f16, name="t", tag="wvall")
        wp_all = cst.tile([128, 4 * DIM], f16, name="t", tag="wpall")
        for big, dram, w in ((xT_all, xT, L), (wq_all, wq, 2 * DIM),
                             (wv_all, wv, DIM), (wp_all, wp, DIM)):
            nc.sync.dma_start(
                big[:].rearrange("p (kc w) -> p kc w", kc=4),
                dram[:].rearrange("(kc p) w -> p kc w", p=128))
        xT_sb = [xT_all[:, i * L:(i + 1) * L] for i in range(4)]
        wq_sb = [wq_all[:, i * 2 * DIM:(i + 1) * 2 * DIM] for i in range(4)]
        wv_sb = [wv_all[:, i * DIM:(i + 1) * DIM] for i in range(4)]
        wp_sb = [wp_all[:, i * DIM:(i + 1) * DIM] for i in range(4)]
        cos_sb = cst.tile([128, 8 * L], f16, name="t", tag="cos")
        sin_sb = cst.tile([128, 8 * L], f16, name="t", tag="sin")
        bias_sb = cst.tile([128, 8], f32, name="t", tag="bias")
        nc.sync.dma_start(cos_sb[:], cosT[:])
        nc.sync.dma_start(sin_sb[:], sinT[:])
        nc.sync.dma_start(bias_sb[:], bias[:])

        # persistent working tiles
        qraw = cst.tile([128, 8 * L], f16, name="t", tag="qraw")
        qsw = cst.tile([128, 8 * L], f16, name="t", tag="qsw")
        # qkTp: 8 blocks of [128, L].  Block m<4 holds RoPE'd q of head
        # pair m (even head dims in rows 0:64, odd head in rows 64:128);
        # block 4+p holds k of pair p the same way.  Scores contract over
        # K=64 row groups via tile_position, so both halves carry data.
        qkTp = cst.tile([128, 8 * L], f16, name="t", tag="qkTp")
        # vaug[kc]: [128 keys, NH*128]; head block h: even h -> v in cols
        # 0:64 and ones in 64:128, odd h -> ones in 0:64 and v in 64:128, so
        # each head's attention numerator lands on the partition rows its
        # slot in the output layout needs (matmul stationary APs must be 2D).
        vaug = [cst.tile([128, NH * 128], f16, name="t", tag=f"vaug{i}") for i in range(8)]
        outT = [cst.tile([128, L], f16, name="t", tag=f"outT{c}") for c in range(4)]

        yall0 = ysb.tile([128, 8 * DIM], f32, name="t", tag="yall")
        nc.vector.memset(yall0[:], 0.0)
        for lb in range(8):
            v3 = vaug[lb][:].rearrange("p (h2 c) -> p h2 c", h2=4)
            nc.vector.memset(v3[:, :, 64:128], 1.0)   # even-head ones
            nc.vector.memset(v3[:, :, 128:192], 1.0)  # odd-head ones

        def drain_copy(dst, src_ap, kind="qk"):
            # act: all drains on ScalarE; dve: all on VectorE;
            # mix: qk (RoPE critical path) on ScalarE, rest on VectorE;
            # split: free-dim halves go to ScalarE and VectorE in parallel
            if _DRAINS == "split" and kind in ("qk", "y"):
                half = dst.shape[-1] // 2
                nc.scalar.copy(dst[..., :half], src_ap[..., :half])
                nc.vector.tensor_copy(dst[..., half:], src_ap[..., half:])
                return
            use_act = (_DRAINS == "act") or (
                _DRAINS == "mix" and kind == "qk") or (
                _DRAINS == "split" and kind != "v2")
            if use_act:
                nc.scalar.copy(dst, src_ap)
            else:
                nc.vector.tensor_copy(dst, src_ap)

        def emit_body(rep):
            # All PSUM pools hold at most 4 banks (2-bank tiles, bufs=2) so
            # adjacent phases can coexist in PSUM and engine pipelines never
            # drain at phase boundaries.
            # ---------- phase 1: QK projection, per-m-block RoPE pipeline ----
            # Drains go to ACT (idle during phase 1); the RoPE swizzle is a
            # per-block SBUF->SBUF DMA; muls/adds per block on DVE so the
            # chain overlaps the next block's matmuls and phase 2 can start
            # as soon as the early head-pair blocks are done.
            with tc.tile_pool(name=f"qkps{rep}", bufs=2, space="PSUM") as qk_ps:
                for m in range(8):
                    ps = qk_ps.tile([128, L], f32, name="t", tag="qkps")
                    for kc in range(4):
                        for qb in range(2):
                            nc.tensor.matmul(
                                ps[:, qb * 512:(qb + 1) * 512],
                                wq_sb[kc][:, m * 128:(m + 1) * 128],
                                xT_sb[kc][:, qb * 512:(qb + 1) * 512],
                                start=(kc == 0), stop=(kc == 3))
                    mc = slice(m * L, (m + 1) * L)
                    drain_copy(qraw[:, mc], ps[:])
                    # swizzle on DVE: an SBUF->SBUF DMA here showed a rare
                    # nondeterministic race (NaNs); DVE copies are cheap
                    # (bf16 4x mode) and DVE has slack under ACT's exp wall
                    for (do, so) in ((0, 32), (32, 0), (64, 96), (96, 64)):
                        nc.vector.tensor_copy(qsw[do:do + 32, mc],
                                              qraw[so:so + 32, mc])
                    nc.vector.tensor_mul(qraw[:, mc], qraw[:, mc], cos_sb[:, mc])
                    nc.vector.tensor_mul(qsw[:, mc], qsw[:, mc], sin_sb[:, mc])
                    nc.vector.tensor_add(qkTp[:, mc], qraw[:, mc], qsw[:, mc])

            # ---------- phase 1b: V projection ----------
            with tc.tile_pool(name=f"vps{rep}", bufs=2, space="PSUM") as v_ps:
                for w in range(4):
                    vps = v_ps.tile([128, 2 * DIM], f32, name="t", tag="vps")
                    for li in range(2):
                        lb = 2 * w + li
                        for kc in range(4):
                            nc.tensor.matmul(
                                vps[:, li * DIM:(li + 1) * DIM],
                                xT_sb[kc][:, lb * 128:(lb + 1) * 128],
                                wv_sb[kc][:],
                                start=(kc == 0), stop=(kc == 3))
                    for li in range(2):
                        lb = 2 * w + li
                        v3 = vaug[lb][:].rearrange("p (h2 c) -> p h2 c", h2=4)
                        p3 = vps[:, li * DIM:(li + 1) * DIM].rearrange(
                            "p (h2 c) -> p h2 c", h2=4)
                        drain_copy(v3[:, :, 0:64], p3[:, :, 0:64], 'v')
                        drain_copy(v3[:, :, 192:256], p3[:, :, 64:128], 'v2')

            # ---------- phase 2: attention ----------
            # 1-kb score groups in 2-bank f32 tiles, double-buffered: PE's
            # scores for group g+1 run during ACT's exp of group g.  AV for
            # group g-1 is emitted after scores g so PE stays busy through
            # the exp pipeline.  X double-buffered so the next head's AV
            # overlaps this head's softmax divide on DVE.
            with tc.tile_pool(name=f"sps{rep}", bufs=2, space="PSUM") as s_ps, \
                 tc.tile_pool(name=f"avps{rep}", bufs=2, space="PSUM") as av_ps:
                for p in range(4 if _PH >= 2 else 0):
                    # head pair (2p, 2p+1): q in qkTp block p, k in block
                    # 4+p; even head rows 0:64, odd rows 64:128.  The two
                    # K=64 score matmuls target different PE row groups
                    # (tile_position) and run concurrently on hardware.
                    qcol = p * L
                    kcol = (4 + p) * L
                    Xe = av_ps.tile([128, L], f32, name="t", tag="avX")
                    Xo = av_ps.tile([128, L], f32, name="t", tag="avX")

                    def emit_scores(u):
                        kb, qb = u >> 1, u & 1
                        s = s_ps.tile([128, L], f32, name="t", tag="s")
                        nc.tensor.matmul(
                            s[:, 0:512],
                            qkTp[0:64, kcol + kb * 128:kcol + (kb + 1) * 128],
                            qkTp[0:64, qcol + qb * 512:qcol + (qb + 1) * 512],
                            start=True, stop=True, tile_position=(0, 0))
                        if _PAIRPOS:
                            nc.tensor.matmul(
                                s[:, 512:1024],
                                qkTp[64:128, kcol + kb * 128:kcol + (kb + 1) * 128],
                                qkTp[64:128, qcol + qb * 512:qcol + (qb + 1) * 512],
                                start=True, stop=True, tile_position=(64, 0))
                        else:
                            # timing control: same data row group as the even
                            # head (wrong numerics, identical instruction mix)
                            nc.tensor.matmul(
                                s[:, 512:1024],
                                qkTp[0:64, kcol + kb * 128:kcol + (kb + 1) * 128],
                                qkTp[0:64, qcol + qb * 512:qcol + (qb + 1) * 512],
                                start=True, stop=True, tile_position=(0, 0))
                        pt = pTp.tile([128, L], f16, name="t", tag="pT")
                        if not _NOEXP:
                            nc.scalar.activation(pt[:], s[:], AF.Exp,
                                                 bias=bias_sb[:, kb:kb + 1],
                                                 scale=SCALE)
                        return pt

                    def emit_av(u, pt):
                        if _PH < 3:
                            return
                        kb, qb = u >> 1, u & 1
                        e, o = 2 * p, 2 * p + 1
                        nc.tensor.matmul(
                            Xe[:, qb * 512:(qb + 1) * 512],
                            vaug[kb][:, e * 128:(e + 1) * 128],
                            pt[:, 0:512],
                            start=(kb == 0), stop=(kb == 7))
                        nc.tensor.matmul(
                            Xo[:, qb * 512:(qb + 1) * 512],
                            vaug[kb][:, o * 128:(o + 1) * 128],
                            pt[:, 512:1024],
                            start=(kb == 0), stop=(kb == 7))

                    prev = emit_scores(0)
                    for u in range(1, 16):
                        cur = emit_scores(u)
                        emit_av(u - 1, prev)
                        prev = cur
                    emit_av(15, prev)

                    if _PH < 3:
                        continue
                    # numerator rows match the output slot per head parity;
                    # recip needs an SBUF-staged input (custom-DVE op
                    # misreads PSUM), hence the D copy.
                    # one [128, L] reciprocal serves both heads of the pair
                    # (DVE time depends on free dim only, not partitions)
                    D = sc.tile([128, L], f32, name="t", tag="D")
                    R = sc.tile([128, L], f32, name="t", tag="R")
                    nc.vector.tensor_copy(D[0:64, :], Xe[64:128, :])
                    nc.vector.tensor_copy(D[64:128, :], Xo[0:64, :])
                    nc.vector.reciprocal_approx_fast(R[:], D[:])
                    nc.vector.tensor_mul(outT[p][0:64, :], Xe[0:64, :], R[0:64, :])
                    nc.vector.tensor_mul(outT[p][64:128, :], Xo[64:128, :],
                                         R[64:128, :])

            # ---------- phase 3: output projection ----------
            # The c<3 partial accumulations only need the early head
            # pairs' outT, so emit them for two waves before any c=3
            # matmul: PE isn't queue-blocked on the last pair's softmax
            # divide until almost all proj work is done.
            with tc.tile_pool(name=f"yps{rep}", bufs=2, space="PSUM") as y_ps:
                yall = yall0

                def proj_partial(w):
                    yp = y_ps.tile([128, 2 * DIM], f32, name="t", tag="yps")
                    for li in range(2):
                        lb = 2 * w + li
                        for c in range(3):
                            nc.tensor.matmul(
                                yp[:, li * DIM:(li + 1) * DIM],
                                outT[c][:, lb * 128:(lb + 1) * 128],
                                wp_sb[c][:],
                                start=(c == 0), stop=False)
                    return yp

                def proj_finish(w, yp):
                    for li in range(2):
                        lb = 2 * w + li
                        nc.tensor.matmul(
                            yp[:, li * DIM:(li + 1) * DIM],
                            outT[3][:, lb * 128:(lb + 1) * 128],
                            wp_sb[3][:],
                            start=False, stop=True)
                    drain_copy(yall[:, 2 * w * DIM:(2 * w + 2) * DIM], yp[:], 'y')

                if _PH >= 4:
                    prev_w, prev_yp = 0, proj_partial(0)
                    for w in range(1, 4):
                        yp = proj_partial(w)
                        proj_finish(prev_w, prev_yp)
                        prev_w, prev_yp = w, yp
                    proj_finish(3, prev_yp)
                    nc.sync.dma_start(
                        y[:].rearrange("(lb p) d -> p lb d", p=128),
                        yall[:].rearrange("p (lb d) -> p lb d", lb=8))

        for rep in range(_REPS):
            emit_body(rep)

    nc.compile()
    return nc


def _rope_tables():
    inv_freq = 1.0 / (10000.0 ** (np.arange(0, HD, 2, dtype=np.float32) / HD))
    t = np.arange(L, dtype=np.float32)
    freqs = np.outer(t, inv_freq)                      # (L, 32)
    emb = np.concatenate([freqs, freqs], axis=-1)      # (L, 64)
    cos = np.cos(emb).T                                # (64, L)
    sin = np.sin(emb).T                                # (64, L)
    sign = np.where(np.arange(HD) < HD // 2, -1.0, 1.0)[:, None].astype(np.float32)
    sin_s = sin * sign
    cosT = np.tile(cos, (2, 1)).astype(np.float16)     # (128, L)
    sinT = np.tile(sin_s, (2, 1)).astype(np.float16)   # (128, L)
    # wide tables: the same [128, L] block tiled across all 8 m-blocks
    return np.tile(cosT, (1, 8)), np.tile(sinT, (1, 8))


_NC = None


def _get_nc():
    global _NC
    if _NC is None:
        _NC = _build_nc()
    return _NC


def _make_in_maps(x, mask, w_qkv, w_proj):
    x = np.asarray(x, dtype=np.float32)
    mask = np.asarray(mask)
    w_qkv = np.asarray(w_qkv, dtype=np.float32)
    w_proj = np.asarray(w_proj, dtype=np.float32)

    cosT, sinT = _rope_tables()
    wq = np.ascontiguousarray(w_qkv[:, :2 * DIM]).astype(np.float16)
    wv = np.ascontiguousarray(w_qkv[:, 2 * DIM:]).astype(np.float16)
    wp = w_proj.astype(np.float16)

    in_maps = []
    for b in range(NCORES):
        xTb = np.ascontiguousarray(x[b].T).astype(np.float16)      # (512, 1024)
        bias_b = np.where(mask[b].reshape(8, 128).T, 0.0, -1e9).astype(np.float32)
        in_maps.append({
            "xT": xTb, "wq": wq, "wv": wv, "wp": wp,
            "cosT": cosT, "sinT": sinT, "bias": bias_b,
        })
    return in_maps


def kernel(x, mask, w_qkv, w_proj):
    nc = _get_nc()
    in_maps = _make_in_maps(x, mask, w_qkv, w_proj)

    from concourse.bass_utils import run_bass_kernel_spmd
    res = run_bass_kernel_spmd(nc, in_maps, core_ids=list(range(NCORES)))
    out = np.stack([res.results[c]["y"] for c in range(NCORES)], axis=0)
    return out.astype(np.float32)



# revision 31
# speedup vs baseline: 1.3208x; 1.3208x over previous
"""Multi-head self-attention (RoPE) Trainium2 Bass kernel.

Shards batch (B=8) across 8 NeuronCores, one batch element per core.

Design notes (HW-A/B-validated):
- Head-PAIRED scores: q/k for heads (2p, 2p+1) live in the two 64-row
  halves of one qkTp block; the two K=64 score matmuls target disjoint
  PE row groups (tile_position (0,0)/(64,0)) and run concurrently —
  measured ~2x scores throughput vs serial K=128 (22us vs 56us/rep for
  the scores-only phases).
- ScalarE is the critical engine (exp is ACT-only at 1x rate,
  ~(N+352)/1.2GHz): exp runs as 64x [128,1024] ACTs off double-buffered
  2-bank score tiles so PE is never gated on a full-group exp.  PSUM
  drains stay on ScalarE — moving them to VectorE measured WORSE
  (RoPE/softmax latency chains live on DVE).
- All PSUM pools are <=4 banks (2-bank tiles, bufs=2) so adjacent
  phases coexist in PSUM; full-8-bank pools cost ~27us/rep in
  phase-boundary stalls.
- RoPE is pipelined per m-block: ACT drain -> SBUF->SBUF DMA swizzle
  -> DVE mul/mul/add, overlapping the next block's matmuls.
- Softmax denominators come free as ones-columns in the augmented V
  (output-partition waste only); one [128,L] reciprocal serves a whole
  head pair.
"""
import os
import sys

# The kernel needs the 8 axon-tunneled NeuronCores visible to jax; a
# JAX_PLATFORMS=cpu pin (used by some harnesses for the reference) would
# hide them. Clear it before jax initializes through the concourse imports.
os.environ.pop("JAX_PLATFORMS", None)

sys.path.insert(0, "/opt/trn_rl_repo")

_REPS = int(os.environ.get("KREPS", "1"))
_DRAINS = os.environ.get("KDRAINS", "act")  # act|dve: engine for PSUM drains
_PH = int(os.environ.get("KPHASES", "4"))  # 1=QKV 2=+scores 3=+AV 4=full
_NOEXP = int(os.environ.get("KNOEXP", "0"))  # 1: skip exp ACTs (cost attribution only)
_PAIRPOS = int(os.environ.get("KPAIRPOS", "1"))  # 0: both score MMs on row group 0 (timing control)

import numpy as np
from contextlib import ExitStack

import concourse.bass as bass
import concourse.tile as tile
from concourse import bacc, mybir

f32 = mybir.dt.float32
f16 = mybir.dt.float16
AF = mybir.ActivationFunctionType
ALU = mybir.AluOpType

B, L, DIM = 8, 1024, 512
NH, HD = 8, 64
SCALE = HD ** -0.5
NCORES = 8


def _build_nc():
    nc = bacc.Bacc("TRN2", target_bir_lowering=False, debug=False, enable_asserts=False)

    xT = nc.dram_tensor("xT", (DIM, L), f16, kind="ExternalInput")
    wq = nc.dram_tensor("wq", (DIM, 2 * DIM), f16, kind="ExternalInput")  # Q|K cols
    wv = nc.dram_tensor("wv", (DIM, DIM), f16, kind="ExternalInput")      # V cols
    wp = nc.dram_tensor("wp", (DIM, DIM), f16, kind="ExternalInput")
    cosT = nc.dram_tensor("cosT", (128, 8 * L), f16, kind="ExternalInput")
    sinT = nc.dram_tensor("sinT", (128, 8 * L), f16, kind="ExternalInput")
    bias = nc.dram_tensor("bias", (128, 8), f32, kind="ExternalInput")
    y = nc.dram_tensor("y", (L, DIM), f32, kind="ExternalOutput")

    with ExitStack() as ctx:
        tc = ctx.enter_context(tile.TileContext(nc))
        cst = ctx.enter_context(tc.tile_pool(name="cst", bufs=1))
        sc = ctx.enter_context(tc.tile_pool(name="sc", bufs=2))
        pTp = ctx.enter_context(tc.tile_pool(name="pTp", bufs=8))
        ysb = ctx.enter_context(tc.tile_pool(name="ysb", bufs=1))

        # ---- load inputs (once) ----
        xT_all = cst.tile([128, 4 * L], f16, name="t", tag="xTall")
        wq_all = cst.tile([128, 4 * 2 * DIM], f16, name="t", tag="wqall")
        wv_all = cst.tile([128, 4 * DIM], f16, name="t", tag="wvall")
        wp_all = cst.tile([128, 4 * DIM], f16, name="t", tag="wpall")
        for big, dram, w in ((xT_all, xT, L), (wq_all, wq, 2 * DIM),
                             (wv_all, wv, DIM), (wp_all, wp, DIM)):
            nc.sync.dma_start(
                big[:].rearrange("p (kc w) -> p kc w", kc=4),
                dram[:].rearrange("(kc p) w -> p kc w", p=128))
        xT_sb = [xT_all[:, i * L:(i + 1) * L] for i in range(4)]
        wq_sb = [wq_all[:, i * 2 * DIM:(i + 1) * 2 * DIM] for i in range(4)]
        wv_sb = [wv_all[:, i * DIM:(i + 1) * DIM] for i in range(4)]
        wp_sb = [wp_all[:, i * DIM:(i + 1) * DIM] for i in range(4)]
        cos_sb = cst.tile([128, 8 * L], f16, name="t", tag="cos")
        sin_sb = cst.tile([128, 8 * L], f16, name="t", tag="sin")
        bias_sb = cst.tile([128, 8], f32, name="t", tag="bias")
        nc.sync.dma_start(cos_sb[:], cosT[:])
        nc.sync.dma_start(sin_sb[:], sinT[:])
        nc.sync.dma_start(bias_sb[:], bias[:])

        # persistent working tiles
        qraw = cst.tile([128, 8 * L], f16, name="t", tag="qraw")
        qsw = cst.tile([128, 8 * L], f16, name="t", tag="qsw")
        # qkTp: 8 blocks of [128, L].  Block m<4 holds RoPE'd q of head
        # pair m (even head dims in rows 0:64, odd head in rows 64:128);
        # block 4+p holds k of pair p the same way.  Scores contract over
        # K=64 row groups via tile_position, so both halves carry data.
        qkTp = cst.tile([128, 8 * L], f16, name="t", tag="qkTp")
        # vaug[kc]: [128 keys, NH*128]; head block h: even h -> v in cols
        # 0:64 and ones in 64:128, odd h -> ones in 0:64 and v in 64:128, so
        # each head's attention numerator lands on the partition rows its
        # slot in the output layout needs (matmul stationary APs must be 2D).
        vaug = [cst.tile([128, NH * 128], f16, name="t", tag=f"vaug{i}") for i in range(8)]
        outT = [cst.tile([128, L], f16, name="t", tag=f"outT{c}") for c in range(4)]

        yall0 = ysb.tile([128, 8 * DIM], f32, name="t", tag="yall")
        nc.vector.memset(yall0[:], 0.0)
        for lb in range(8):
            v3 = vaug[lb][:].rearrange("p (h2 c) -> p h2 c", h2=4)
            nc.vector.memset(v3[:, :, 64:128], 1.0)   # even-head ones
            nc.vector.memset(v3[:, :, 128:192], 1.0)  # odd-head ones

        def drain_copy(dst, src_ap, kind="qk"):
            # act: all drains on ScalarE; dve: all on VectorE;
            # mix: qk (RoPE critical path) on ScalarE, rest on VectorE;
            # split: free-dim halves go to ScalarE and VectorE in parallel
            if _DRAINS == "split" and kind in ("qk", "y"):
                half = dst.shape[-1] // 2
                nc.scalar.copy(dst[..., :half], src_ap[..., :half])
                nc.vector.tensor_copy(dst[..., half:], src_ap[..., half:])
                return
            use_act = (_DRAINS == "act") or (
                _DRAINS == "mix" and kind == "qk") or (
                _DRAINS == "split" and kind != "v2")
            if use_act:
                nc.scalar.copy(dst, src_ap)
            else:
                nc.vector.tensor_copy(dst, src_ap)

        def emit_body(rep):
            # All PSUM pools hold at most 4 banks (2-bank tiles, bufs=2) so
            # adjacent phases can coexist in PSUM and engine pipelines never
            # drain at phase boundaries.
            # ---------- phase 1: QK projection, per-m-block RoPE pipeline ----
            # Drains go to ACT (idle during phase 1); the RoPE swizzle is a
            # per-block SBUF->SBUF DMA; muls/adds per block on DVE so the
            # chain overlaps the next block's matmuls and phase 2 can start
            # as soon as the early head-pair blocks are done.
            with tc.tile_pool(name=f"qkps{rep}", bufs=2, space="PSUM") as qk_ps:
                for m in range(8):
                    ps = qk_ps.tile([128, L], f32, name="t", tag="qkps")
                    for kc in range(4):
                        for qb in range(2):
                            nc.tensor.matmul(
                                ps[:, qb * 512:(qb + 1) * 512],
                                wq_sb[kc][:, m * 128:(m + 1) * 128],
                                xT_sb[kc][:, qb * 512:(qb + 1) * 512],
                                start=(kc == 0), stop=(kc == 3))
                    mc = slice(m * L, (m + 1) * L)
                    drain_copy(qraw[:, mc], ps[:])
                    # swizzle on DVE: an SBUF->SBUF DMA here showed a rare
                    # nondeterministic race (NaNs); DVE copies are cheap
                    # (bf16 4x mode) and DVE has slack under ACT's exp wall
                    for (do, so) in ((0, 32), (32, 0), (64, 96), (96, 64)):
                        nc.vector.tensor_copy(qsw[do:do + 32, mc],
                                              qraw[so:so + 32, mc])
                    nc.vector.tensor_mul(qraw[:, mc], qraw[:, mc], cos_sb[:, mc])
                    nc.vector.tensor_mul(qsw[:, mc], qsw[:, mc], sin_sb[:, mc])
                    nc.vector.tensor_add(qkTp[:, mc], qraw[:, mc], qsw[:, mc])

            # ---------- phase 1b: V projection ----------
            with tc.tile_pool(name=f"vps{rep}", bufs=2, space="PSUM") as v_ps:
                for w in range(4):
                    vps = v_ps.tile([128, 2 * DIM], f32, name="t", tag="vps")
                    for li in range(2):
                        lb = 2 * w + li
                        for kc in range(4):
                            nc.tensor.matmul(
                                vps[:, li * DIM:(li + 1) * DIM],
                                xT_sb[kc][:, lb * 128:(lb + 1) * 128],
                                wv_sb[kc][:],
                                start=(kc == 0), stop=(kc == 3))
                    for li in range(2):
                        lb = 2 * w + li
                        v3 = vaug[lb][:].rearrange("p (h2 c) -> p h2 c", h2=4)
                        p3 = vps[:, li * DIM:(li + 1) * DIM].rearrange(
                            "p (h2 c) -> p h2 c", h2=4)
                        drain_copy(v3[:, :, 0:64], p3[:, :, 0:64], 'v')
                        drain_copy(v3[:, :, 192:256], p3[:, :, 64:128], 'v2')

            # ---------- phase 2: attention ----------
            # 1-kb score groups in 2-bank f32 tiles, double-buffered: PE's
            # scores for group g+1 run during ACT's exp of group g.  AV for
            # group g-1 is emitted after scores g so PE stays busy through
            # the exp pipeline.  X double-buffered so the next head's AV
            # overlaps this head's softmax divide on DVE.
            with tc.tile_pool(name=f"sps{rep}", bufs=2, space="PSUM") as s_ps, \
                 tc.tile_pool(name=f"avps{rep}", bufs=2, space="PSUM") as av_ps:
                for p in range(4 if _PH >= 2 else 0):
                    # head pair (2p, 2p+1): q in qkTp block p, k in block
                    # 4+p; even head rows 0:64, odd rows 64:128.  The two
                    # K=64 score matmuls target different PE row groups
                    # (tile_position) and run concurrently on hardware.
                    qcol = p * L
                    kcol = (4 + p) * L
                    Xe = av_ps.tile([128, L], f32, name="t", tag="avX")
                    Xo = av_ps.tile([128, L], f32, name="t", tag="avX")

                    def emit_scores(u):
                        kb, qb = u >> 1, u & 1
                        s = s_ps.tile([128, L], f32, name="t", tag="s")
                        nc.tensor.matmul(
                            s[:, 0:512],
                            qkTp[0:64, kcol + kb * 128:kcol + (kb + 1) * 128],
                            qkTp[0:64, qcol + qb * 512:qcol + (qb + 1) * 512],
                            start=True, stop=True, tile_position=(0, 0))
                        if _PAIRPOS:
                            nc.tensor.matmul(
                                s[:, 512:1024],
                                qkTp[64:128, kcol + kb * 128:kcol + (kb + 1) * 128],
                                qkTp[64:128, qcol + qb * 512:qcol + (qb + 1) * 512],
                                start=True, stop=True, tile_position=(64, 0))
                        else:
                            # timing control: same data row group as the even
                            # head (wrong numerics, identical instruction mix)
                            nc.tensor.matmul(
                                s[:, 512:1024],
                                qkTp[0:64, kcol + kb * 128:kcol + (kb + 1) * 128],
                                qkTp[0:64, qcol + qb * 512:qcol + (qb + 1) * 512],
                                start=True, stop=True, tile_position=(0, 0))
                        pt = pTp.tile([128, L], f16, name="t", tag="pT")
                        if not _NOEXP:
                            nc.scalar.activation(pt[:], s[:], AF.Exp,
                                                 bias=bias_sb[:, kb:kb + 1],
                                                 scale=SCALE)
                        return pt

                    def emit_av(u, pt):
                        if _PH < 3:
                            return
                        kb, qb = u >> 1, u & 1
                        e, o = 2 * p, 2 * p + 1
                        nc.tensor.matmul(
                            Xe[:, qb * 512:(qb + 1) * 512],
                            vaug[kb][:, e * 128:(e + 1) * 128],
                            pt[:, 0:512],
                            start=(kb == 0), stop=(kb == 7))
                        nc.tensor.matmul(
                            Xo[:, qb * 512:(qb + 1) * 512],
                            vaug[kb][:, o * 128:(o + 1) * 128],
                            pt[:, 512:1024],
                            start=(kb == 0), stop=(kb == 7))

                    prev = emit_scores(0)
                    for u in range(1, 16):
                        cur = emit_scores(u)
                        emit_av(u - 1, prev)
                        prev = cur
                    emit_av(15, prev)

                    if _PH < 3:
                        continue
                    # numerator rows match the output slot per head parity;
                    # recip needs an SBUF-staged input (custom-DVE op
                    # misreads PSUM), hence the D copy.
                    # one [128, L] reciprocal serves both heads of the pair
                    # (DVE time depends on free dim only, not partitions)
                    D = sc.tile([128, L], f32, name="t", tag="D")
                    R = sc.tile([128, L], f32, name="t", tag="R")
                    nc.vector.tensor_copy(D[0:64, :], Xe[64:128, :])
                    nc.vector.tensor_copy(D[64:128, :], Xo[0:64, :])
                    nc.vector.reciprocal_approx_fast(R[:], D[:])
                    nc.vector.tensor_mul(outT[p][0:64, :], Xe[0:64, :], R[0:64, :])
                    nc.vector.tensor_mul(outT[p][64:128, :], Xo[64:128, :],
                                         R[64:128, :])

            # ---------- phase 3: output projection ----------
            # The c<3 partial accumulations only need the early head
            # pairs' outT, so emit them for two waves before any c=3
            # matmul: PE isn't queue-blocked on the last pair's softmax
            # divide until almost all proj work is done.
            with tc.tile_pool(name=f"yps{rep}", bufs=2, space="PSUM") as y_ps:
                yall = yall0

                def proj_partial(w):
                    yp = y_ps.tile([128, 2 * DIM], f32, name="t", tag="yps")
                    for li in range(2):
                        lb = 2 * w + li
                        for c in range(3):
                            nc.tensor.matmul(
                                yp[:, li * DIM:(li + 1) * DIM],
                                outT[c][:, lb * 128:(lb + 1) * 128],
                                wp_sb[c][:],
                                start=(c == 0), stop=False)
                    return yp

                y3 = y[:].rearrange("(lb p) d -> p lb d", p=128)
                yall3 = yall[:].rearrange("p (lb d) -> p lb d", lb=8)

                def proj_finish(w, yp):
                    for li in range(2):
                        lb = 2 * w + li
                        nc.tensor.matmul(
                            yp[:, li * DIM:(li + 1) * DIM],
                            outT[3][:, lb * 128:(lb + 1) * 128],
                            wp_sb[3][:],
                            start=False, stop=True)
                    drain_copy(yall[:, 2 * w * DIM:(2 * w + 2) * DIM], yp[:], 'y')
                    # ship this wave's rows immediately: the output DMA for
                    # wave w overlaps the remaining waves' matmuls/drains
                    nc.sync.dma_start(y3[:, 2 * w:2 * w + 2, :],
                                      yall3[:, 2 * w:2 * w + 2, :])

                if _PH >= 4:
                    prev_w, prev_yp = 0, proj_partial(0)
                    for w in range(1, 4):
                        yp = proj_partial(w)
                        proj_finish(prev_w, prev_yp)
                        prev_w, prev_yp = w, yp
                    proj_finish(3, prev_yp)

        for rep in range(_REPS):
            emit_body(rep)

    nc.compile()
    return nc


def _rope_tables():
    inv_freq = 1.0 / (10000.0 ** (np.arange(0, HD, 2, dtype=np.float32) / HD))
    t = np.arange(L, dtype=np.float32)
    freqs = np.outer(t, inv_freq)                      # (L, 32)
    emb = np.concatenate([freqs, freqs], axis=-1)      # (L, 64)
    cos = np.cos(emb).T                                # (64, L)
    sin = np.sin(emb).T                                # (64, L)
    sign = np.where(np.arange(HD) < HD // 2, -1.0, 1.0)[:, None].astype(np.float32)
    sin_s = sin * sign
    cosT = np.tile(cos, (2, 1)).astype(np.float16)     # (128, L)
    sinT = np.tile(sin_s, (2, 1)).astype(np.float16)   # (128, L)
    # wide tables: the same [128, L] block tiled across all 8 m-blocks
    return np.tile(cosT, (1, 8)), np.tile(sinT, (1, 8))


_NC = None


def _get_nc():
    global _NC
    if _NC is None:
        _NC = _build_nc()
    return _NC


def _make_in_maps(x, mask, w_qkv, w_proj):
    x = np.asarray(x, dtype=np.float32)
    mask = np.asarray(mask)
    w_qkv = np.asarray(w_qkv, dtype=np.float32)
    w_proj = np.asarray(w_proj, dtype=np.float32)

    cosT, sinT = _rope_tables()
    wq = np.ascontiguousarray(w_qkv[:, :2 * DIM]).astype(np.float16)
    wv = np.ascontiguousarray(w_qkv[:, 2 * DIM:]).astype(np.float16)
    wp = w_proj.astype(np.float16)

    in_maps = []
    for b in range(NCORES):
        xTb = np.ascontiguousarray(x[b].T).astype(np.float16)      # (512, 1024)
        bias_b = np.where(mask[b].reshape(8, 128).T, 0.0, -1e9).astype(np.float32)
        in_maps.append({
            "xT": xTb, "wq": wq, "wv": wv, "wp": wp,
            "cosT": cosT, "sinT": sinT, "bias": bias_b,
        })
    return in_maps


def kernel(x, mask, w_qkv, w_proj):
    nc = _get_nc()
    in_maps = _make_in_maps(x, mask, w_qkv, w_proj)

    from concourse.bass_utils import run_bass_kernel_spmd
    res = run_bass_kernel_spmd(nc, in_maps, core_ids=list(range(NCORES)))
    out = np.stack([res.results[c]["y"] for c in range(NCORES)], axis=0)
    return out.astype(np.float32)

